# revision 16
# baseline (speedup 1.0000x reference)
"""GRU cell kernel for Trainium2, data-parallel across 8 NeuronCores.

Reference computation (per batch row):
    concat = [h_prev, x]                       # [B, 2048]
    z = sigmoid(concat @ W_z.T + b_z)          # [B, 1024]
    r = sigmoid(concat @ W_r.T + b_r)
    h_tilde = tanh([r*h_prev, x] @ W_h.T + b_h)
    h = (1-z)*h_prev + z*h_tilde
Device computes dh = z*(h_tilde - h_prev); h = h_prev + dh on the host.

Sharding: batch dim (8192) split 1024/core; weights replicated.
Matmuls are fp8-e4m3 perf_mode=DoubleRow; the steady stream measures
~216ns per [256K x 512N] chunk = the fp8 moving-port roofline, so all
remaining time is head (DMA spin-up) and tail (epilogue + exit barrier).

Head design (from ntff analysis of the previous schedule):
  - The framework preamble runs to ~7.0us; first DMA descriptors issue
    then, first payload packets land ~8.5-9.2us. Each engine ring
    trickles ~80-160 GB/s at the head (per-packet latency bound), so
    the earliest sustainable PE stream start is ~11.5us.
  - Every head-critical transfer is its own CONTIGUOUS dram tensor:
    a DMA whose DRAM side is one contiguous block gets 4-8KB packets
    (cross-partition coalescing); slicing a bigger tensor gives
    per-partition-run packets (1KB) and a ~2x slower trickle.
  - wr0/wr1 are split kk-major (h-half / x-half) so the ramp's first
    LDWEIGHTS depends on a 128KB block, not 256KB, and spread across
    the gpsimd ring ahead of the gated bulk. h8/x8 acts are split per
    (n-half, ko-group) across sync (n0) and vector (n1) rings; x8n1
    rides scalar. hb rides sync/vector right behind the fp8 acts so
    finish-r can free the 4 ramp PSUM banks by ~20us.
  - The warmup (clock-ramp) matmul count is sized so the PE runs out
    of dummy work right when wr0A+h8n0a land (~11.4us); the HAM clock
    gate needs ~3.3us of busy to reach 2.4GHz, and a stall mid-stream
    drops it back (costs ~2x the gap).
Tail: finish("h") out-DMAs rotate across sync/gpsimd/vector rings so
descriptor issue isn't serialized on one engine, and the last chains
split (nchain/nsub) so the final piece's ACT+mul+DMA is small.
"""

import numpy as np

import concourse.bacc as bacc
import concourse.bass as bass
import concourse.mybir as mybir
import concourse.tile as tile
from concourse import bass_utils

P = 128
B = 8192
I = 1024
H = 1024
K = I + H            # 2048 contraction
NCORES = 8
BS = B // NCORES     # 1024 batch rows per core
MT = H // P          # 8 m-tiles (hidden units)
KT = K // P          # 16 k-chunks of 128
KK = K // (2 * P)    # 8 double-chunks of 256 (DoubleRow)
NFREE = 512          # moving free dim (one PSUM bank of fp32)
NT = BS // NFREE     # 2 n-tiles per core
KO = 8               # feature chunks per 1024-feature tensor
WS = 512.0           # host-side weight scale for fp8 range
NWARM = 22           # clock-ramp dummy matmuls (tuned to data arrival)

F32 = mybir.dt.float32
BF16 = mybir.dt.bfloat16
F8 = mybir.dt.float8e4

AF = mybir.ActivationFunctionType
DR = mybir.MatmulPerfMode.DoubleRow


def build_kernel(mode: str = "fp8h"):
    """Build the per-core Bass kernel. Returns compiled nc."""
    assert mode == "fp8h"
    nc = bacc.Bacc("TRN2", target_bir_lowering=False, debug=False)

    # DRAM I/O. Each head-critical transfer is its own contiguous tensor.
    def dt(name, shape, dtype):
        return nc.dram_tensor(name, shape, dtype, kind="ExternalInput").ap()

    # fp8 act pieces: 4 ko-pair pieces per (tensor, n-half) so arrival is
    # progressive and each ramp matmul depends on a single 128KB block.
    ap = {}
    for t in ("h8", "x8"):
        for n in range(NT):
            for p in range(4):
                ap[t, n, p] = dt(f"{t}n{n}p{p}", [P, 2, NFREE], F8)
    hb0 = dt("hb0", [P, KO, NFREE], BF16)
    hb1 = dt("hb1", [P, KO, NFREE], BF16)
    wq = {}
    for g in ("r", "z"):
        for m in (0, 1):
            wq[g, m, 0] = dt(f"w{g}{m}q0", [P, KT // 4, P], F8)
            wq[g, m, 1] = dt(f"w{g}{m}q1", [P, KT // 4, P], F8)
            wq[g, m, 2] = dt(f"w{g}{m}B", [P, KT // 2, P], F8)
    Wr = dt("Wr", [MT, P, K], F8)      # only m>=2 transferred
    Wz = dt("Wz", [MT, P, K], F8)
    Wh = dt("Wh", [MT, P, K], F8)
    bz = dt("bz", [P, MT], F32)
    br = dt("br", [P, MT], F32)
    bh = dt("bh", [P, MT], F32)
    out = nc.dram_tensor("out", [H, BS], BF16, kind="ExternalOutput").ap()

    with tile.TileContext(nc) as tc:
        with (
            tc.tile_pool(name="acts", bufs=1) as acts,
            tc.tile_pool(name="gates", bufs=1) as gates,
            tc.tile_pool(name="wpool", bufs=1) as wpool,
            tc.tile_pool(name="opool", bufs=10) as opool,
            tc.tile_pool(name="ppool", bufs=8, space="PSUM") as ppool,
        ):
            bz_sb = acts.tile([P, MT], F32)
            br_sb = acts.tile([P, MT], F32)
            bh_sb = acts.tile([P, MT], F32)

            # Weight tiles, [P, KT, P]: [:, 2k:2k+2, :] is a DoubleRow
            # stationary operand [128, 2, 128].
            wr_sb = [wpool.tile([P, KT, P], F8, name=f"wr{m}")
                     for m in range(MT)]
            wz_sb = [wpool.tile([P, KT, P], F8, name=f"wz{m}")
                     for m in range(MT)]
            wh_sb = [wpool.tile([P, KT, P], F8, name=f"wh{m}")
                     for m in range(MT)]

            # Persistent activations: [p, n-half, ko, bw]
            x8_sb = acts.tile([P, NT, KO, NFREE], F8)
            h8_sb = acts.tile([P, NT, KO, NFREE], F8)
            hb_sb = acts.tile([P, NT, KO, NFREE], BF16)

            # Warmup operand: zeroed fp8 tile, memset first so the PE can
            # start clock-ramp matmuls as soon as the preamble ends.
            zt = acts.tile([P, 2, NFREE // 2], F8)
            warm = acts.tile([P, 1], F32)
            nc.vector.memset(zt[:], 0.0)
            nc.vector.memset(warm[:], 0.0)

            # ---- Head DMA schedule: 3 rings, ordered by PE need-time ----
            # The two HWDGE rings (sync/scalar, ~0.7-1.5us completion-sem
            # latency) carry the ramp-critical prefix interleaved in exact
            # need order: n0 data + mt0 weight quarters on sync, n1 data +
            # mt1 quarters on scalar. The gpsimd SWDGE ring (sems lag
            # +1-2us) carries only bulk whose need-time has slack.
            gsb = {"r": wr_sb, "z": wz_sb}

            def act_dma(eng, t, n, p):
                sb = h8_sb if t == "h8" else x8_sb
                eng.dma_start(sb[:, n, 2 * p:2 * p + 2, :], ap[t, n, p])

            def wq_dma(eng, g, m, q):
                eng.dma_start(
                    gsb[g][m][:, q * KT // 4:(q + 1) * KT // 4, :],
                    wq[g, m, q])

            for n, eng in ((0, nc.sync), (1, nc.scalar)):
                act_dma(eng, "h8", n, 0)
                wq_dma(eng, "r", n, 0)
                wq_dma(eng, "z", n, 0)
                act_dma(eng, "h8", n, 1)
                wq_dma(eng, "r", n, 1)
                wq_dma(eng, "z", n, 1)
                if n == 1:
                    eng.dma_start(br_sb[:], br)
                act_dma(eng, "h8", n, 2)
                act_dma(eng, "h8", n, 3)
                act_dma(eng, "x8", n, 0)
                act_dma(eng, "x8", n, 1)
                act_dma(eng, "x8", n, 2)
                act_dma(eng, "x8", n, 3)
            nc.sync.dma_start(hb_sb[:, 0], hb0)
            nc.scalar.dma_start(bz_sb[:], bz)
            nc.scalar.dma_start(bh_sb[:], bh)
            nc.scalar.dma_start(hb_sb[:, 1], hb1)
            # gpsimd ring: x-part halves of the ramp weights (needed from
            # ~18us; SWDGE sem lag is fine), then the gated bulk. The gate
            # copy reads an x8-n0 tail slice, so the tile framework holds
            # the bulk descriptors until the sync ring's ramp-critical act
            # DMAs have landed (SDMA engines round-robin across queues at
            # packet granularity; un-gated bulk would steal head bandwidth
            # from the acts).
            for g in ("r", "z"):
                for m in (0, 1):
                    nc.gpsimd.dma_start(gsb[g][m][:, KT // 2:, :],
                                        wq[g, m, 2])
            dma_gate = opool.tile([P, 8], F8, name="dma_gate")
            nc.gpsimd.tensor_copy(dma_gate[:], x8_sb[:, 0, 6, 0:8])
            for m in range(2, MT):
                nc.gpsimd.dma_start(wr_sb[m][:], Wr[m])
                nc.gpsimd.dma_start(wz_sb[m][:], Wz[m])
            for m in range(MT):
                nc.gpsimd.dma_start(wh_sb[m][:], Wh[m])

            # Pre-warm both ACT tables while the DMA window runs (the
            # table loads ride the static queue and don't block Scalar's
            # descriptor issues for long).
            nc.scalar.activation(warm[:], warm[:], AF.Sigmoid)
            nc.scalar.activation(warm[:], warm[:], AF.Tanh)

            # Gate results, same swizzled layout
            z_sb = gates.tile([P, NT, KO, NFREE], BF16)
            rh_sb = gates.tile([P, NT, KO, NFREE], F8)

            # Clock-ramp warmup: dummy matmuls on the zeroed tile while the
            # first weights/acts stream in (HAM gate: 1.2 -> 2.4 GHz needs
            # ~3.3us busy). Sized to end right at data arrival (~11.4us).
            ps_wf = ppool.tile([P, NFREE], F32, tag="ps", name="ps_warm")
            for i in range(NWARM):
                nc.tensor.matmul(ps_wf[:, 0:NFREE // 2], zt[:, :, 0:P], zt[:],
                                 start=(i == 0), stop=(i == NWARM - 1),
                                 perf_mode=DR)

            def rz_rhs(kk, n):
                """fp8 moving operand [128,2,512] for concat chunk kk."""
                if kk < KK // 2:
                    return h8_sb[:, n, 2 * kk:2 * kk + 2, :]
                c = kk - KK // 2
                return x8_sb[:, n, 2 * c:2 * c + 2, :]

            def h_rhs(kk, n):
                """fp8 moving operand for the h-gate ([r*h_prev, x])."""
                if kk < KK // 2:
                    return rh_sb[:, n, 2 * kk:2 * kk + 2, :]
                c = kk - KK // 2
                return x8_sb[:, n, 2 * c:2 * c + 2, :]

            # out-writes rotate across the two HWDGE rings (sync/scalar);
            # the gpsimd SWDGE queue has ~1us extra latency and a slow
            # exit drain, so it must not carry tail traffic.
            OUT_ENGS = [nc.sync, nc.scalar]
            out_idx = [0]

            def finish(stage, mt, n, ps, width=NFREE, sub=0):
                """PSUM -> activation -> elementwise -> (store)."""
                lo, hi = sub * width, (sub + 1) * width
                if stage == "r":
                    r_tmp = opool.tile([P, width], BF16, tag="rt")
                    nc.scalar.activation(r_tmp, ps, AF.Sigmoid,
                                         bias=br_sb[:, mt:mt + 1],
                                         scale=1.0 / WS)
                    nc.vector.tensor_mul(
                        rh_sb[:, n, mt, lo:hi], r_tmp, hb_sb[:, n, mt, lo:hi])
                elif stage == "z":
                    nc.scalar.activation(z_sb[:, n, mt, lo:hi], ps,
                                         AF.Sigmoid,
                                         bias=bz_sb[:, mt:mt + 1],
                                         scale=1.0 / WS)
                else:
                    # device computes dh = z*(h_tilde - h_prev); the final
                    # h = h_prev + dh runs on the host in fp32 (free, and
                    # keeps the dominant h_prev term exact).
                    ht = opool.tile([P, width], BF16, tag="ht")
                    nc.scalar.activation(ht, ps, AF.Tanh,
                                         bias=bh_sb[:, mt:mt + 1],
                                         scale=1.0 / WS)
                    nc.vector.tensor_sub(ht, ht, hb_sb[:, n, mt, lo:hi])
                    nc.vector.tensor_mul(ht, ht, z_sb[:, n, mt, lo:hi])
                    ns = slice(n * NFREE + lo, n * NFREE + hi)
                    eng = OUT_ENGS[out_idx[0] % len(OUT_ENGS)]
                    out_idx[0] += 1
                    eng.dma_start(out[mt * P:(mt + 1) * P, ns], ht)

            def chain(stage, w_sb, rhs, mt, n, nsub=1, nchain=1):
                """One (mt, n) PSUM accumulation chain + its epilogue.

                nchain>1 splits the matmuls into narrower column chains so
                the epilogue of chain c pipelines under chain c+1's matmuls
                (used near the kernel tail to shrink the exit latency).
                """
                wc = NFREE // nchain
                for c in range(nchain):
                    psf = ppool.tile([P, NFREE], F32, tag="ps",
                                     name=f"ps_{stage}{mt}_{n}_{c}")
                    ps = psf[:, 0:wc]
                    for kk in range(KK):
                        nc.tensor.matmul(
                            ps, w_sb[mt][:, 2 * kk:2 * kk + 2, :],
                            rhs(kk, n)[:, :, c * wc:(c + 1) * wc],
                            start=(kk == 0), stop=(kk == KK - 1),
                            perf_mode=DR)
                    w2 = wc // nsub
                    for s in range(nsub):
                        finish(stage, mt, n, ps[:, s * w2:(s + 1) * w2],
                               width=w2, sub=c * nsub + s)

            # 8-chain ramp: r AND z gates for mt0/mt1 x both batch halves
            # as open PSUM chains. The head is DMA-bandwidth bound, so the
            # ramp maximizes reuse per delivered byte: each h8/x8 chunk is
            # read by 8 matmuls (vs 2 in steady state), halving the byte
            # rate the rings must sustain before the bulk pipeline deepens.
            # Order: r h-part (kk0-3), z h-part, then x-part per chain
            # (r n0, r n1, z n0, z n1) so chains close early and free
            # their PSUM banks for the steady phase.
            psr = {(g, mt, n): ppool.tile([P, NFREE], F32, tag="ps",
                                          name=f"ps_{g}{mt}_{n}")
                   for g in ("r", "z") for mt in (0, 1) for n in range(NT)}

            def ramp_mm(g, kk, n, mt):
                nc.tensor.matmul(
                    psr[(g, mt, n)], gsb[g][mt][:, 2 * kk:2 * kk + 2, :],
                    rz_rhs(kk, n),
                    start=(kk == 0), stop=(kk == KK - 1), perf_mode=DR)

            for g in ("r", "z"):
                for kk in range(KK // 2):
                    for n in range(NT):
                        for mt in (0, 1):
                            ramp_mm(g, kk, n, mt)
            for g in ("r", "z"):
                for n in range(NT):
                    for mt in (0, 1):
                        for kk in range(KK // 2, KK):
                            ramp_mm(g, kk, n, mt)
            for g in ("r", "z"):
                for mt in (0, 1):
                    for n in range(NT):
                        finish(g, mt, n, psr[(g, mt, n)])
            for mt in range(2, MT):
                for n in range(NT):
                    chain("r", wr_sb, rz_rhs, mt, n)
                for n in range(NT):
                    chain("z", wz_sb, rz_rhs, mt, n)
            for mt in range(MT):
                for n in range(NT):
                    last2 = mt == MT - 1
                    last1 = last2 and n == NT - 1
                    chain("h", wh_sb, h_rhs, mt, n,
                          nchain=2 if (last2 or (mt == MT - 2 and n == 1))
                          else 1,
                          nsub=2 if last1 else 1)

    nc.compile()
    return nc


def _prep_inputs(x, h_prev, W_z, b_z, W_r, b_r, W_h, b_h, mode="fp8h"):
    """Host-side relayout: swizzled feature-major acts, m-tiled weights.

    Every device transfer is a separate contiguous array so the DMA
    engines coalesce 4-8KB packets (see module docstring).
    """
    import ml_dtypes
    F8NP = ml_dtypes.float8_e4m3fn
    BFNP = ml_dtypes.bfloat16

    def prep_w(W, dt):
        # w[mt, p, ko*128+m] = W[mt*128+m, ko*128+p], scaled for fp8 range
        MTl, Kl = W.shape[0] // P, W.shape[1]
        W4 = (W * WS).reshape(MTl, P, Kl // P, P)      # [mt, m, ko, p]
        return np.ascontiguousarray(
            W4.transpose(0, 3, 2, 1)).reshape(MTl, P, Kl).astype(dt)

    def prep_act(aT, dt):
        # [F, bs] -> [p, n, ko, bw]
        return np.ascontiguousarray(
            aT.reshape(KO, P, NT, NFREE).transpose(1, 2, 0, 3)).astype(dt)

    def prep_b(b):
        return np.ascontiguousarray(b.reshape(MT, P).T)

    xT = np.ascontiguousarray(x.T)                         # [I, B] f32
    hT = np.ascontiguousarray(h_prev.T)                    # [H, B] f32

    wr = prep_w(W_r, F8NP)                                 # [MT, P, K]
    wz = prep_w(W_z, F8NP)
    shared = {
        "Wr": wr, "Wz": wz, "Wh": prep_w(W_h, F8NP),
        "bz": prep_b(b_z), "br": prep_b(b_r), "bh": prep_b(b_h),
    }
    for g, w in (("r", wr), ("z", wz)):
        w4 = w.reshape(MT, P, KT, P)
        for m in (0, 1):
            shared[f"w{g}{m}q0"] = np.ascontiguousarray(w4[m, :, 0:KT // 4])
            shared[f"w{g}{m}q1"] = np.ascontiguousarray(
                w4[m, :, KT // 4:KT // 2])
            shared[f"w{g}{m}B"] = np.ascontiguousarray(w4[m, :, KT // 2:])
    in_maps = []
    for c in range(NCORES):
        bs = slice(c * BS, (c + 1) * BS)
        x8 = prep_act(xT[:, bs], F8NP)                     # [P, NT, KO, BW]
        h8 = prep_act(hT[:, bs], F8NP)
        hb = prep_act(hT[:, bs], BFNP)
        m = dict(shared)
        for t, a in (("h8", h8), ("x8", x8)):
            for n in range(NT):
                for p in range(4):
                    m[f"{t}n{n}p{p}"] = np.ascontiguousarray(
                        a[:, n, 2 * p:2 * p + 2])
        m["hb0"] = np.ascontiguousarray(hb[:, 0])
        m["hb1"] = np.ascontiguousarray(hb[:, 1])
        in_maps.append(m)
    return in_maps


def run(inputs, mode="fp8h", trace=False, **run_kwargs):
    """Compile + run on 8 cores. Returns (output [B,H] f32, results)."""
    run_kwargs.pop("mm_dtype", None)
    nc = build_kernel(mode)
    in_maps = _prep_inputs(**inputs, mode=mode)
    res = bass_utils.run_bass_kernel_spmd(
        nc, in_maps, core_ids=list(range(NCORES)), trace=trace, **run_kwargs)
    dhT = np.concatenate(
        [res.results[c]["out"] for c in range(NCORES)], axis=1)  # [H, B] bf16
    dh = np.ascontiguousarray(dhT.T).astype(np.float32)
    return inputs["h_prev"] + dh, res


def kernel(**inputs) -> np.ndarray:
    import time as _time
    try:
        out, _ = run(inputs)
    except Exception:
        # The axon-tunneled device occasionally reports a transient
        # "unrecoverable" state right after a crashed session; a fresh
        # attempt after a short pause recovers.
        _time.sleep(15)
        out, _ = run(inputs)
    return out
